# revision 1
# baseline (speedup 1.0000x reference)
"""Trainium2 Bass kernel for nn_DiTXMoEBlock (DiT block: adaLN + self-attn +
gated cross-attn + top-2-of-8 MoE FFN + shared expert).

Strategy (8 NeuronCores, full inputs in / full output out):
- Pass A (data-parallel, 512 query tokens per core = half a batch):
  everything up to h2 (the modulated LN before the MoE), feature-major
  [C, T] activations, all matmuls in float32r (full PE speed at N>=256,
  ~11-bit mantissa => no routing flips vs the fp32 reference).
- Host: router softmax + top-2 in fp32, token->expert chunking (32 chunks
  of <=512 tokens: expert chunks + shared-expert chunks), gather.
- Pass B (expert-parallel, 4 chunk-slots of 512 tokens per core): the
  routed expert FFNs and the shared expert as generic (W1,W2,b1,b2) slots,
  bf16 matmuls, exact-erf GELU on the scalar engine.
- Host: weighted scatter-add combine + final residual.
"""

import numpy as np

try:
    import concourse.bacc as bacc
except ImportError:  # fall back to the repo checkout location
    import sys
    sys.path.insert(0, "/opt/trn_rl_repo")
    import concourse.bacc as bacc

import ml_dtypes
import concourse.bass as bass
import concourse.mybir as mybir
from concourse.bass_utils import run_bass_kernel_spmd
from concourse.tile import TileContext

AF = mybir.ActivationFunctionType
ALU = mybir.AluOpType
f32 = mybir.dt.float32
f32r = mybir.dt.float32r
bf16 = mybir.dt.bfloat16

B, N, L, C, H, E, TOPK, F = 4, 1024, 512, 768, 12, 8, 2, 3072
D = C // H          # 64
P = 128             # partitions
T = 512             # tokens per core in pass A (half a batch)
CT = C // P         # 6 C-tiles
FT = F // P         # 24 F-tiles
NKT = N // P        # 8 key tiles (self-attention, full batch seq)
LKT = L // P        # 4 key tiles (cross-attention context)
EPS = 1e-5
NSLOT = 4           # expert-chunk slots per core in pass B
CAP = 512           # tokens per chunk slot
VW = 65             # per-head width in v_tm layout (64 v cols + 1 ones col)

_CACHE = {}


# --------------------------------------------------------------------------
# Pass A kernel builder
# --------------------------------------------------------------------------

def _build_pass_a():
    nc = bacc.Bacc("TRN2", target_bir_lowering=False, debug=False, num_devices=8)

    din = {}
    for nm, shape in [
        ("xT", [C, T]), ("xoT", [C, T]), ("cT", [C, T]),
        ("Wq_sa", [C, C]), ("Wk_sa", [C, C]), ("Wv_sa", [C, C]),
        ("Wqq", [C, C]), ("Wqg", [C, C]), ("Wk_ca", [C, C]), ("Wv_ca", [C, C]),
        ("Wp_sa", [C, C]), ("Wp_ca", [C, C]),
        ("rows2", [2, C]),          # bp_sa, bp_ca
        ("onesr", [1, T]), ("onesc", [P, 1]),
        ("vinit", [P, H * VW]),     # zeros with 1.0 at col 64 of each 65-block
    ]:
        din[nm] = nc.dram_tensor(nm, shape, f32r, kind="ExternalInput")
    din["cols"] = nc.dram_tensor("cols", [P, 7 * CT], f32, kind="ExternalInput")
    x3_out = nc.dram_tensor("x3T", [C, T], f32r, kind="ExternalOutput")
    h2_out = nc.dram_tensor("h2T", [C, T], f32r, kind="ExternalOutput")

    with TileContext(nc) as tc, \
         nc.allow_low_precision("float32r rounding of matmul operands is intended"):
        with tc.tile_pool(name="acts", bufs=1) as acts, \
             tc.tile_pool(name="wpool", bufs=8) as wpool, \
             tc.tile_pool(name="vec", bufs=1) as vecp, \
             tc.tile_pool(name="sq", bufs=2) as sqp, \
             tc.tile_pool(name="ps_big", bufs=2, space="PSUM") as ps_big, \
             tc.tile_pool(name="ps_sc", bufs=2, space="PSUM") as ps_sc, \
             tc.tile_pool(name="ps_bc", bufs=1, space="PSUM") as ps_bc, \
             tc.tile_pool(name="ps_st", bufs=1, space="PSUM") as ps_st:

            evict_ctr = [0]

            def evict_copy(dst_ap, src_ap):
                # alternate psum->sbuf copies between ACT and DVE
                if evict_ctr[0] % 2 == 0:
                    nc.scalar.copy(dst_ap, src_ap)
                else:
                    nc.vector.tensor_copy(dst_ap, src_ap)
                evict_ctr[0] += 1

            # --- small constants / vectors ---
            ones_row = vecp.tile([1, T], f32r, tag="ones_row")
            nc.sync.dma_start(ones_row[:, :], din["onesr"][:, :])
            ones_col = vecp.tile([P, 1], f32r, tag="ones_col")
            nc.sync.dma_start(ones_col[:, :], din["onesc"][:, :])
            eps_t = vecp.tile([1, 1], f32, tag="eps")
            nc.vector.memset(eps_t[:, :], EPS)
            cols = vecp.tile([P, 7 * CT], f32, tag="cols")
            nc.sync.dma_start(cols[:, :], din["cols"][:, :])
            c_sc1m, c_shm = cols[:, 0:CT], cols[:, CT:2 * CT]
            c_gam, c_bet = cols[:, 2 * CT:3 * CT], cols[:, 3 * CT:4 * CT]
            c_sc1f, c_shf = cols[:, 4 * CT:5 * CT], cols[:, 5 * CT:6 * CT]
            gcol = cols[:, 6 * CT:7 * CT]
            r_bpsa = vecp.tile([1, C], f32r, tag="row_bpsa")
            nc.sync.dma_start(r_bpsa[:, :], din["rows2"][0:1, :])
            r_bpca = vecp.tile([1, C], f32r, tag="row_bpca")
            nc.sync.dma_start(r_bpca[:, :], din["rows2"][1:2, :])

            # --- load activations ---
            def load_fm(name, tag, ncols=T):
                ts = []
                for i in range(CT):
                    t_ = acts.tile([P, ncols], f32r, tag=f"{tag}{i}")
                    nc.sync.dma_start(t_[:, :], din[name][i * P:(i + 1) * P, :])
                    ts.append(t_)
                return ts

            x_t = load_fm("xT", "x")
            xo_t = load_fm("xoT", "xo")
            c_t = load_fm("cT", "c")

            def load_w(name):
                ws = []
                for ki in range(CT):
                    w = wpool.tile([P, C], f32r, tag="wblk")
                    nc.sync.dma_start(w[:, :], din[name][ki * P:(ki + 1) * P, :])
                    ws.append(w)
                return ws

            # --- LayerNorm helpers ---
            def ln_stats(src_tiles, label):
                """Return (a, b) [1,T] f32r tiles: a = rstd, b = -mean*rstd."""
                st_x = ps_st.tile([1, T], f32, tag="st")
                for i in range(CT):
                    nc.tensor.matmul(st_x[:, :], ones_col[:, :], src_tiles[i][:, :],
                                     start=(i == 0), stop=(i == CT - 1))
                mean = vecp.tile([1, T], f32, tag="mean", bufs=1)
                nc.vector.tensor_scalar_mul(mean[:, :], st_x[:, :], 1.0 / C)
                st_xx = ps_st.tile([1, T], f32, tag="st")
                for i in range(CT):
                    sq = sqp.tile([P, T], f32r, tag="sq")
                    nc.scalar.square(sq[:, :], src_tiles[i][:, :])
                    nc.tensor.matmul(st_xx[:, :], ones_col[:, :], sq[:, :],
                                     start=(i == 0), stop=(i == CT - 1))
                mean2 = vecp.tile([1, T], f32, tag="mean2", bufs=1)
                nc.vector.tensor_mul(mean2[:, :], mean[:, :], mean[:, :])
                var = vecp.tile([1, T], f32, tag="var", bufs=1)
                nc.vector.scalar_tensor_tensor(var[:, :], st_xx[:, :], 1.0 / C,
                                               mean2[:, :], ALU.mult, ALU.subtract)
                sd = vecp.tile([1, T], f32, tag="sd", bufs=1)
                nc.scalar.activation(sd[:, :], var[:, :], AF.Sqrt,
                                     bias=eps_t[:, 0:1])
                a = vecp.tile([1, T], f32r, tag="a", bufs=1)
                nc.vector.reciprocal(a[:, :], sd[:, :])
                b = vecp.tile([1, T], f32r, tag="b", bufs=1)
                nc.vector.scalar_tensor_tensor(b[:, :], mean[:, :], -1.0, a[:, :],
                                               ALU.mult, ALU.mult)
                return a, b

            def ln_apply(src_tiles, a, b, scale_cols, shift_cols, out_tag,
                         out_tiles=None):
                """out = (ln(src)) * scale[c] + shift[c], feature-major.

                scale_cols/shift_cols: [P, CT] f32 column packs (or None).
                """
                outs = []
                # the broadcasts are identical for every C-tile: compute once
                bcA = ps_bc.tile([P, T], f32, tag="bcA")
                nc.tensor.matmul(bcA[:, :], ones_row[:, 0:P], a[:, :],
                                 start=True, stop=True)
                bcB = ps_bc.tile([P, T], f32, tag="bcB")
                nc.tensor.matmul(bcB[:, :], ones_row[:, 0:P], b[:, :],
                                 start=True, stop=True)
                for i in range(CT):
                    if out_tiles is not None:
                        o = out_tiles[i]
                    else:
                        o = acts.tile([P, T], f32r, tag=f"{out_tag}{i}")
                    nc.vector.tensor_mul(o[:, :], src_tiles[i][:, :], bcA[:, :])
                    nc.vector.tensor_add(o[:, :], o[:, :], bcB[:, :])
                    if scale_cols is not None:
                        nc.vector.tensor_scalar(o[:, :], o[:, :],
                                                scale_cols[:, i:i + 1],
                                                shift_cols[:, i:i + 1],
                                                ALU.mult, ALU.add)
                    outs.append(o)
                return outs

            # --- LN1 + modulate (own + other half), LN on context ---
            a_x, b_x = ln_stats(x_t, "x")
            h_t = ln_apply(x_t, a_x, b_x, c_sc1m, c_shm, "h")
            a_xo, b_xo = ln_stats(xo_t, "xo")
            ho_t = ln_apply(xo_t, a_xo, b_xo, c_sc1m, c_shm, "ho")
            a_c, b_c = ln_stats(c_t, "c")
            cm_t = ln_apply(c_t, a_c, b_c, c_gam, c_bet, "cm")

            # --- linear helper: out_fm[oi] += W[:, oi].T @ act ---
            def linear_fm(w_tiles, act_tiles, out_tag, out_dtype=f32r,
                          bias_row=None, n_out=CT, evict=None):
                outs = []
                for oi in range(n_out):
                    ps = ps_big.tile([P, T], f32, tag="big")
                    for ki in range(CT):
                        nc.tensor.matmul(ps[:, :],
                                         w_tiles[ki][:, oi * P:(oi + 1) * P],
                                         act_tiles[ki][:, :],
                                         start=(ki == 0),
                                         stop=(ki == CT - 1 and bias_row is None))
                    if bias_row is not None:
                        nc.tensor.matmul(ps[:, :], bias_row[:, oi * P:(oi + 1) * P],
                                         ones_row[:, :], start=False, stop=True)
                    if evict is not None:
                        outs.append(evict(oi, ps))
                    else:
                        o = acts.tile([P, T], out_dtype, tag=f"{out_tag}{oi}")
                        evict_copy(o[:, :], ps[:, :])
                        outs.append(o)
                return outs

            # --- q/k/v projections ---
            w_q = load_w("Wq_sa")
            q_t = linear_fm(w_q, h_t, "q")

            w_k = load_w("Wk_sa")
            k_t = []
            for oi in range(CT):
                kt_ = acts.tile([P, N], f32r, tag=f"k{oi}")
                k_t.append(kt_)
            for half, act in ((0, h_t), (1, ho_t)):
                for oi in range(CT):
                    ps = ps_big.tile([P, T], f32, tag="big")
                    for ki in range(CT):
                        nc.tensor.matmul(ps[:, :], w_k[ki][:, oi * P:(oi + 1) * P],
                                         act[ki][:, :], start=(ki == 0),
                                         stop=(ki == CT - 1))
                    evict_copy(k_t[oi][:, half * T:(half + 1) * T], ps[:, :])

            # v in token-major interleaved layout: v_tm[kt][:, 65h:65h+64] = v,
            # col 65h+64 = ones  (for fused row-sum in the AV matmul)
            w_v = load_w("Wv_sa")
            v_tm = []
            for kt in range(NKT):
                vt = acts.tile([P, H * VW], f32r, tag=f"vtm{kt}")
                v_tm.append(vt)
                nc.sync.dma_start(vt[:, :], din["vinit"][:, :])
            for kt in range(NKT):
                act = h_t if kt < 4 else ho_t
                tj = kt % 4
                for ch in range(2):  # two 384-wide output chunks (6 heads each)
                    ps = ps_big.tile([P, 384], f32, tag="big")
                    for ki in range(CT):
                        nc.tensor.matmul(
                            ps[:, :],
                            act[ki][:, tj * P:(tj + 1) * P],
                            w_v[ki][:, ch * 384:(ch + 1) * 384],
                            start=(ki == 0), stop=(ki == CT - 1))
                    # strided eviction into the interleaved layout
                    vb = v_tm[kt][:, :]
                    dst = bass.AP(vb.tensor, vb.offset + ch * 6 * VW,
                                  [list(vb.ap[0]), [VW, 6], [1, 64]])
                    evict_copy(dst, ps[:, :].rearrange("p (h d) -> p h d", h=6))
                evict_ctr[0] += 1

            # --- generic attention: scores^T -> exp -> AV (+fused softmax norm) ---
            def attention(q_tiles, k_tiles, v_tm_tiles, nkt, out_tag):
                """Returns normalized attention output, feature-major [CT x [P,T]]."""
                out_tiles = [acts.tile([P, T], f32r, tag=f"{out_tag}{i}",
                                       name=f"{out_tag}{i}")
                             for i in range(CT)]
                for h in range(H):
                    th, ro = h // 2, 64 * (h % 2)
                    exps = []
                    for kt in range(nkt):
                        sps = ps_sc.tile([P, T], f32, tag="score")
                        nc.tensor.matmul(
                            sps[:, :],
                            k_tiles[th][ro:ro + 64, kt * P:(kt + 1) * P],
                            q_tiles[th][ro:ro + 64, :],
                            start=True, stop=True)
                        ex = acts.tile([P, T], f32r, tag=f"exp{kt}")
                        nc.scalar.activation(ex[:, :], sps[:, :], AF.Exp,
                                             scale=float(D ** -0.5))
                        exps.append(ex)
                    avps = ps_big.tile([VW, T], f32, tag="av", bufs=1)
                    for kt in range(nkt):
                        nc.tensor.matmul(avps[:, :],
                                         v_tm_tiles[kt][:, h * VW:(h + 1) * VW],
                                         exps[kt][:, :],
                                         start=(kt == 0), stop=(kt == nkt - 1))
                    # softmax denominator: recip of rowsum, broadcast to 64 rows,
                    # multiplied in during the psum eviction
                    srec = vecp.tile([1, T], f32r, tag="srec", bufs=2)
                    nc.vector.reciprocal(srec[:, :], avps[64:65, :])
                    brc = ps_bc.tile([64, T], f32, tag="bcA")
                    nc.tensor.matmul(brc[:, :], ones_row[:, 0:64], srec[:, :],
                                     start=True, stop=True)
                    evict_copy(out_tiles[th][ro:ro + 64, :], avps[0:64, :])
                    nc.vector.tensor_mul(out_tiles[th][ro:ro + 64, :],
                                         out_tiles[th][ro:ro + 64, :], brc[:, :])
                return out_tiles

            sa_t = attention(q_t, k_t, v_tm, NKT, "c")

            # --- proj_sa + gated residual: x2 = x + g_msa * (sa @ Wp + bp) ---
            w_p = load_w("Wp_sa")

            def evict_res_gated(oi, ps):
                nc.vector.scalar_tensor_tensor(x_t[oi][:, :], ps[:, :],
                                               gcol[:, oi:oi + 1], x_t[oi][:, :],
                                               ALU.mult, ALU.add)
                return x_t[oi]
            linear_fm(w_p, sa_t, None, bias_row=r_bpsa, evict=evict_res_gated)
            # x_t now holds x2

            # --- LN2 -> hx; q2 / gate projections ---
            a_2, b_2 = ln_stats(x_t, "x2")
            hx_t = ln_apply(x_t, a_2, b_2, None, None, "h", out_tiles=h_t)

            w_qq = load_w("Wqq")
            q2_t = linear_fm(w_qq, hx_t, "q")   # reuse q tags
            w_qg = load_w("Wqg")
            sig_t = []

            def evict_sig(oi, ps):
                o = acts.tile([P, T], f32, tag=f"xo{oi}")
                nc.scalar.activation(o[:, :], ps[:, :], AF.Sigmoid)
                sig_t.append(o)
                return o
            linear_fm(w_qg, hx_t, None, evict=evict_sig)

            # --- cross-attention K/V from modulated context ---
            w_k2 = load_w("Wk_ca")
            k2_t = linear_fm(w_k2, cm_t, "ho")
            w_v2 = load_w("Wv_ca")
            v2_tm = []
            for kt in range(LKT):
                vt = acts.tile([P, H * VW], f32r, tag=f"vtm{kt}")
                v2_tm.append(vt)
                nc.sync.dma_start(vt[:, :], din["vinit"][:, :])
            for kt in range(LKT):
                for ch in range(2):
                    ps = ps_big.tile([P, 384], f32, tag="big")
                    for ki in range(CT):
                        nc.tensor.matmul(
                            ps[:, :],
                            cm_t[ki][:, kt * P:(kt + 1) * P],
                            w_v2[ki][:, ch * 384:(ch + 1) * 384],
                            start=(ki == 0), stop=(ki == CT - 1))
                    vb = v2_tm[kt][:, :]
                    dst = bass.AP(vb.tensor, vb.offset + ch * 6 * VW,
                                  [list(vb.ap[0]), [VW, 6], [1, 64]])
                    evict_copy(dst, ps[:, :].rearrange("p (h d) -> p h d", h=6))
                evict_ctr[0] += 1

            ca_t = attention(q2_t, k2_t, v2_tm, LKT, "cm")
            for i in range(CT):
                nc.vector.tensor_mul(ca_t[i][:, :], ca_t[i][:, :], sig_t[i][:, :])

            # --- proj_ca + residual: x3 = x2 + (ca @ Wp_ca + bp_ca) ---
            w_p2 = load_w("Wp_ca")

            def evict_res(oi, ps):
                nc.vector.tensor_add(x_t[oi][:, :], ps[:, :], x_t[oi][:, :])
                nc.sync.dma_start(x3_out[oi * P:(oi + 1) * P, :], x_t[oi][:, :])
                return x_t[oi]
            linear_fm(w_p2, ca_t, None, bias_row=r_bpca, evict=evict_res)
            # x_t now holds x3

            # --- LN3 + MLP modulation -> h2 ---
            a_3, b_3 = ln_stats(x_t, "x3")
            h2_t = ln_apply(x_t, a_3, b_3, c_sc1f, c_shf, "h", out_tiles=h_t)
            for i in range(CT):
                nc.sync.dma_start(h2_out[i * P:(i + 1) * P, :], h2_t[i][:, :])

    nc.finalize()
    return nc


# --------------------------------------------------------------------------
# Pass B kernel builder: NSLOT generic expert slots of CAP tokens
# --------------------------------------------------------------------------

def _build_pass_b():
    nc = bacc.Bacc("TRN2", target_bir_lowering=False, debug=False, num_devices=8)

    h2s = nc.dram_tensor("h2s", [NSLOT, C, CAP], bf16, kind="ExternalInput")
    W1s = nc.dram_tensor("W1s", [NSLOT, C, F], bf16, kind="ExternalInput")
    W2s = nc.dram_tensor("W2s", [NSLOT, F, C], bf16, kind="ExternalInput")
    b1c = nc.dram_tensor("b1c", [NSLOT, P, FT], f32, kind="ExternalInput")
    b2c = nc.dram_tensor("b2c", [NSLOT, P, CT], f32, kind="ExternalInput")
    y_out = nc.dram_tensor("y", [NSLOT, C, CAP], f32, kind="ExternalOutput")

    with TileContext(nc) as tc:
        with tc.tile_pool(name="acts", bufs=2) as acts, \
             tc.tile_pool(name="h1p", bufs=FT + 8) as h1p, \
             tc.tile_pool(name="w1p", bufs=8) as w1p, \
             tc.tile_pool(name="w2p", bufs=26) as w2p, \
             tc.tile_pool(name="vec", bufs=2) as vecp, \
             tc.tile_pool(name="ps", bufs=4, space="PSUM") as psp:

            for s in range(NSLOT):
                b1 = vecp.tile([P, FT], f32, tag="b1")
                nc.sync.dma_start(b1[:, :], b1c[s, :, :])
                b2 = vecp.tile([P, CT], f32, tag="b2")
                nc.sync.dma_start(b2[:, :], b2c[s, :, :])
                h2_t = []
                for i in range(CT):
                    t_ = acts.tile([P, CAP], bf16, tag=f"h2_{i}")
                    nc.sync.dma_start(t_[:, :], h2s[s, i * P:(i + 1) * P, :])
                    h2_t.append(t_)
                w1_t = []
                for ki in range(CT):
                    w = w1p.tile([P, F], bf16, tag="w1")
                    nc.sync.dma_start(w[:, :], W1s[s, ki * P:(ki + 1) * P, :])
                    w1_t.append(w)
                # h1 = gelu(h2 @ W1 + b1)
                h1_t = []
                for oj in range(FT):
                    ps = psp.tile([P, CAP], f32, tag="ps")
                    for ki in range(CT):
                        nc.tensor.matmul(ps[:, :], w1_t[ki][:, oj * P:(oj + 1) * P],
                                         h2_t[ki][:, :], start=(ki == 0),
                                         stop=(ki == CT - 1))
                    o = h1p.tile([P, CAP], bf16, tag="h1")
                    nc.scalar.activation(o[:, :], ps[:, :], AF.Gelu,
                                         bias=b1[:, oj:oj + 1])
                    h1_t.append(o)
                # y = h1 @ W2 + b2
                w2_t = []
                for kj in range(FT):
                    w = w2p.tile([P, C], bf16, tag="w2")
                    nc.sync.dma_start(w[:, :], W2s[s, kj * P:(kj + 1) * P, :])
                    w2_t.append(w)
                for oi in range(CT):
                    ps = psp.tile([P, CAP], f32, tag="ps")
                    for kj in range(FT):
                        nc.tensor.matmul(ps[:, :], w2_t[kj][:, oi * P:(oi + 1) * P],
                                         h1_t[kj][:, :], start=(kj == 0),
                                         stop=(kj == FT - 1))
                    o = acts.tile([P, CAP], f32, tag=f"y_{oi}")
                    nc.scalar.add(o[:, :], ps[:, :], b2[:, oi:oi + 1])
                    nc.sync.dma_start(y_out[s, oi * P:(oi + 1) * P, :], o[:, :])

    nc.finalize()
    return nc


def _get_nc(which):
    if which not in _CACHE:
        _CACHE[which] = _build_pass_a() if which == "a" else _build_pass_b()
    return _CACHE[which]


# --------------------------------------------------------------------------
# Host orchestration
# --------------------------------------------------------------------------

def _silu(x):
    return x / (1.0 + np.exp(-x))


def _softmax(x, axis=-1):
    x = x - x.max(axis=axis, keepdims=True)
    e = np.exp(x)
    return e / e.sum(axis=axis, keepdims=True)


def _ln_np(v, eps=EPS):
    m = v.mean(-1, keepdims=True)
    var = v.var(-1, keepdims=True)
    return (v - m) / np.sqrt(var + eps)


def _refine_logits(logits, amb, x, c, mod_vecs, tcond, W_qkv, Wqq, Wqg,
                   W_kv, Wp_sa, bp_sa, Wp_ca, bp_ca, W_router):
    """Recompute router logits exactly (fp32 host) for ambiguous tokens.

    The device pass runs matmuls in float32r (~11-bit mantissa), which is
    enough to route every token whose top-2 margin exceeds ~1e-4. For the
    handful of near-tie tokens, redo the whole block math for just those
    tokens in fp32 so the expert choice matches a full-precision reference.
    """
    f = np.float32
    sh_msa, sc_msa, g_msa, sh_mlp, sc_mlp, g_mlp, gamma, beta = mod_vecs
    scale = f(D) ** -0.5
    for b_ in np.unique(amb // N):
        tloc = amb[amb // N == b_] % N
        hb = _ln_np(x[b_]) * (1.0 + sc_msa[b_]) + sh_msa[b_]      # [N, C]
        k = (hb @ W_qkv[:, C:2 * C]).reshape(N, H, D)
        v = (hb @ W_qkv[:, 2 * C:]).reshape(N, H, D)
        q = (hb[tloc] @ W_qkv[:, :C]).reshape(-1, H, D)
        s = np.einsum('ahd,lhd->ahl', q * scale, k)
        s = np.exp(s - s.max(-1, keepdims=True))
        attn = s / s.sum(-1, keepdims=True)
        sa = np.einsum('ahl,lhd->ahd', attn, v).reshape(-1, C)
        sa = sa @ Wp_sa + bp_sa
        x2a = x[b_, tloc] + g_msa[b_] * sa
        cm = _ln_np(c[b_]) * gamma[b_] + beta[b_]
        k2 = (cm @ W_kv[:, :C]).reshape(L, H, D)
        v2 = (cm @ W_kv[:, C:]).reshape(L, H, D)
        hxa = _ln_np(x2a)
        q2 = (hxa @ Wqq).reshape(-1, H, D)
        gate = (hxa @ Wqg).reshape(-1, H, D)
        s2 = np.einsum('ahd,lhd->ahl', q2 * scale, k2)
        s2 = np.exp(s2 - s2.max(-1, keepdims=True))
        attn2 = s2 / s2.sum(-1, keepdims=True)
        ao = np.einsum('ahl,lhd->ahd', attn2, v2)
        ao = ao * (1.0 / (1.0 + np.exp(-gate)))
        ca = ao.reshape(-1, C) @ Wp_ca + bp_ca
        x3a = x2a + ca
        h2a = _ln_np(x3a) * (1.0 + sc_mlp[b_]) + sh_mlp[b_]
        logits[b_ * N + tloc] = h2a @ W_router + tcond[b_]
    return logits


def kernel(x, c, t, W_ada, b_ada, W_qkv, W_proj_sa, b_proj_sa, W_q, W_kv,
           W_proj_ca, b_proj_ca, W_cadaln, b_cadaln, W_router, W_tcond,
           W1, b1, W2, b2, Ws1, bs1, Ws2, bs2):
    f = np.float32
    x, c, t = np.asarray(x, f), np.asarray(c, f), np.asarray(t, f)

    # ---- host: tiny t-conditioned vectors (per batch) ----
    st = _silu(t)
    mod = st @ np.asarray(W_ada, f) + np.asarray(b_ada, f)          # [B, 6C]
    sh_msa, sc_msa, g_msa, sh_mlp, sc_mlp, g_mlp = np.split(mod, 6, axis=-1)
    gb = st @ np.asarray(W_cadaln, f) + np.asarray(b_cadaln, f)     # [B, 2C]
    gamma, beta = np.split(gb, 2, axis=-1)
    tcond = t @ np.asarray(W_tcond, f)                              # [B, E]

    # ---- pass A inputs ----
    W_qkv = np.asarray(W_qkv, f)
    Wq_sa = np.ascontiguousarray(W_qkv[:, :C])
    Wk_sa = np.ascontiguousarray(W_qkv[:, C:2 * C])
    Wv_sa = np.ascontiguousarray(W_qkv[:, 2 * C:])
    W_q = np.asarray(W_q, f).reshape(C, H, 2 * D)
    Wqq = np.ascontiguousarray(W_q[:, :, :D].reshape(C, C))
    Wqg = np.ascontiguousarray(W_q[:, :, D:].reshape(C, C))
    W_kv = np.asarray(W_kv, f)
    Wk_ca = np.ascontiguousarray(W_kv[:, :C])
    Wv_ca = np.ascontiguousarray(W_kv[:, C:])
    Wp_sa = np.asarray(W_proj_sa, f)
    Wp_ca = np.asarray(W_proj_ca, f)
    bp_sa = np.asarray(b_proj_sa, f)
    bp_ca = np.asarray(b_proj_ca, f)
    rows2 = np.ascontiguousarray(np.stack([bp_sa, bp_ca]))
    onesr = np.ones((1, T), f)
    onesc = np.ones((P, 1), f)
    vinit = np.zeros((P, H * VW), f)
    vinit[:, 64::VW] = 1.0

    in_maps_a = []
    for core in range(8):
        b_, half = core // 2, core % 2
        sl = slice(half * T, (half + 1) * T)
        so = slice((1 - half) * T, (2 - half) * T)
        cols = np.zeros((P, 7 * CT), f)
        for j, v in enumerate([1.0 + sc_msa[b_], sh_msa[b_], gamma[b_],
                               beta[b_], 1.0 + sc_mlp[b_], sh_mlp[b_],
                               g_msa[b_]]):
            cols[:, j * CT:(j + 1) * CT] = v.reshape(CT, P).T
        in_maps_a.append({
            "xT": np.ascontiguousarray(x[b_, sl].T),
            "xoT": np.ascontiguousarray(x[b_, so].T),
            "cT": np.ascontiguousarray(c[b_].T),
            "Wq_sa": Wq_sa, "Wk_sa": Wk_sa, "Wv_sa": Wv_sa,
            "Wqq": Wqq, "Wqg": Wqg, "Wk_ca": Wk_ca, "Wv_ca": Wv_ca,
            "Wp_sa": Wp_sa, "Wp_ca": Wp_ca,
            "rows2": rows2,
            "cols": cols,
            "onesr": onesr, "onesc": onesc, "vinit": vinit,
        })

    nc_a = _get_nc("a")
    res_a = run_bass_kernel_spmd(nc_a, in_maps_a, core_ids=list(range(8)))

    x3 = np.empty((B, N, C), f)
    h2 = np.empty((B, N, C), f)
    for core in range(8):
        b_, half = core // 2, core % 2
        sl = slice(half * T, (half + 1) * T)
        x3[b_, sl] = res_a.results[core]["x3T"].T
        h2[b_, sl] = res_a.results[core]["h2T"].T

    # ---- host: router (fp32) + top-2 + chunking ----
    W_router = np.asarray(W_router, f)
    h2f = h2.reshape(-1, C)
    logits = h2f @ W_router
    logits += np.repeat(tcond, N, axis=0)
    probs = _softmax(logits, axis=-1)
    # near-tie tokens: the device pass's float32r rounding could flip their
    # top-2 choice vs a full-precision reference -- redo just those on host
    ps_sorted = np.sort(probs, axis=-1)
    amb = np.nonzero(ps_sorted[:, -2] - ps_sorted[:, -3] < 2e-3)[0]
    if 0 < len(amb) <= 512:
        mod_vecs = (sh_msa, sc_msa, g_msa, sh_mlp, sc_mlp, g_mlp, gamma, beta)
        logits = _refine_logits(logits, amb, x, c, mod_vecs, tcond, W_qkv,
                                Wqq, Wqg, W_kv, Wp_sa, bp_sa, Wp_ca, bp_ca,
                                W_router)
        probs[amb] = _softmax(logits[amb], axis=-1)
    order = np.argsort(-probs, axis=-1, kind="stable")
    topi = order[:, :TOPK]
    topv = np.take_along_axis(probs, topi, axis=-1)
    topv = topv / topv.sum(-1, keepdims=True)

    W1 = np.asarray(W1, f)
    W2 = np.asarray(W2, f)
    b1 = np.asarray(b1, f)
    b2 = np.asarray(b2, f)
    Ws1 = np.asarray(Ws1, f)
    Ws2 = np.asarray(Ws2, f)
    bs1 = np.asarray(bs1, f)
    bs2 = np.asarray(bs2, f)

    # chunks: (tokens, weights_vec, W1, b1, W2, b2)
    chunks = []
    for e_ in range(E):
        sel = np.nonzero(topi == e_)
        toks = sel[0]
        wv = topv[sel]
        for s0 in range(0, len(toks), CAP):
            chunks.append((toks[s0:s0 + CAP], wv[s0:s0 + CAP],
                           W1[e_], b1[e_], W2[e_], b2[e_]))
    all_toks = np.arange(B * N)
    for s0 in range(0, B * N, CAP):
        tk = all_toks[s0:s0 + CAP]
        chunks.append((tk, np.ones(len(tk), f), Ws1, bs1, Ws2, bs2))
    assert len(chunks) <= 8 * NSLOT, f"too many chunks: {len(chunks)}"
    zW1 = np.zeros((C, F), f)
    zW2 = np.zeros((F, C), f)
    zb1 = np.zeros(F, f)
    zb2 = np.zeros(C, f)
    while len(chunks) < 8 * NSLOT:
        chunks.append((all_toks[:0], np.ones(0, f), zW1, zb1, zW2, zb2))

    h2T = np.ascontiguousarray(h2f.T)  # [C, B*N]
    in_maps_b = []
    for core in range(8):
        h2s = np.zeros((NSLOT, C, CAP), ml_dtypes.bfloat16)
        W1s = np.empty((NSLOT, C, F), ml_dtypes.bfloat16)
        W2s = np.empty((NSLOT, F, C), ml_dtypes.bfloat16)
        b1cs = np.empty((NSLOT, P, FT), f)
        b2cs = np.empty((NSLOT, P, CT), f)
        for s in range(NSLOT):
            toks, _wv, cw1, cb1, cw2, cb2 = chunks[core * NSLOT + s]
            if len(toks):
                h2s[s, :, :len(toks)] = h2T[:, toks].astype(ml_dtypes.bfloat16)
            W1s[s] = cw1.astype(ml_dtypes.bfloat16)
            W2s[s] = cw2.astype(ml_dtypes.bfloat16)
            b1cs[s] = cb1.reshape(FT, P).T
            b2cs[s] = cb2.reshape(CT, P).T
        in_maps_b.append({"h2s": h2s, "W1s": W1s, "W2s": W2s,
                          "b1c": b1cs, "b2c": b2cs})

    nc_b = _get_nc("b")
    res_b = run_bass_kernel_spmd(nc_b, in_maps_b, core_ids=list(range(8)))

    # ---- host: weighted scatter-add combine + final residual ----
    accum = np.zeros((B * N, C), f)
    for core in range(8):
        y = res_b.results[core]["y"]  # [NSLOT, C, CAP]
        for s in range(NSLOT):
            toks, wv, *_ = chunks[core * NSLOT + s]
            if len(toks):
                # tokens are unique within a chunk, so fancy-index += is safe
                accum[toks] += wv[:, None] * y[s, :, :len(toks)].T

    out = x3 + g_mlp[:, None, :] * accum.reshape(B, N, C)
    return out.astype(np.float32)



# revision 7
# speedup vs baseline: 1.4578x; 1.4578x over previous
"""Trainium2 Bass kernel for nn_DiTXMoEBlock (DiT block: adaLN + self-attn +
gated cross-attn + top-2-of-8 MoE FFN + shared expert).

Strategy (8 NeuronCores, full inputs in / full output out):
- Pass A (data-parallel, 512 query tokens per core = half a batch):
  everything up to h2 (the modulated LN before the MoE), feature-major
  [C, T] activations, all matmuls in float32r (full PE speed at N>=256,
  ~11-bit mantissa => no routing flips vs the fp32 reference).
- Host: router softmax + top-2 in fp32, token->expert chunking (32 chunks
  of <=512 tokens: expert chunks + shared-expert chunks), gather.
- Pass B (expert-parallel, 4 chunk-slots of 512 tokens per core): the
  routed expert FFNs and the shared expert as generic (W1,W2,b1,b2) slots,
  bf16 matmuls, exact-erf GELU on the scalar engine.
- Host: weighted scatter-add combine + final residual.
"""

import numpy as np

try:
    import concourse.bacc as bacc
except ImportError:  # fall back to the repo checkout location
    import sys
    sys.path.insert(0, "/opt/trn_rl_repo")
    import concourse.bacc as bacc

import ml_dtypes
import concourse.bass as bass
import concourse.mybir as mybir
from concourse.bass_utils import run_bass_kernel_spmd
from concourse.tile import TileContext

AF = mybir.ActivationFunctionType
ALU = mybir.AluOpType
f32 = mybir.dt.float32
f32r = mybir.dt.float32r
bf16 = mybir.dt.bfloat16

B, N, L, C, H, E, TOPK, F = 4, 1024, 512, 768, 12, 8, 2, 3072
D = C // H          # 64
P = 128             # partitions
T = 512             # tokens per core in pass A (half a batch)
CT = C // P         # 6 C-tiles
FT = F // P         # 24 F-tiles
NKT = N // P        # 8 key tiles (self-attention, full batch seq)
LKT = L // P        # 4 key tiles (cross-attention context)
EPS = 1e-5
NSLOT = 4           # expert-chunk slots per core in pass B
CAP = 512           # tokens per chunk slot
VW = 65             # per-head width in v_tm layout (64 v cols + 1 ones col)

_CACHE = {}
LAST_B_KEY = None   # set by kernel(): pass-B variant used on the last call


# --------------------------------------------------------------------------
# Pass A kernel builder
# --------------------------------------------------------------------------

def _build_pass_a():
    nc = bacc.Bacc("TRN2", target_bir_lowering=False, debug=False, num_devices=8)

    din = {}
    for nm, shape in [
        ("xT", [C, T]), ("xoT", [C, T]), ("cT", [C, T]),
        ("Wq_sa", [C, C]), ("Wk_sa", [C, C]), ("Wv_sa", [C, C]),
        ("Wqq", [C, C]), ("Wqg", [C, C]), ("Wk_ca", [C, C]), ("Wv_ca", [C, C]),
        ("Wp_sa", [C, C]), ("Wp_ca", [C, C]),
        ("rows2", [2, C]),          # bp_sa, bp_ca
        ("onesr", [1, T]), ("onesc", [P, 1]),
        ("vinit", [P, H * VW]),     # zeros with 1.0 at col 64 of each 65-block
    ]:
        din[nm] = nc.dram_tensor(nm, shape, f32r, kind="ExternalInput")
    din["cols"] = nc.dram_tensor("cols", [P, 7 * CT], f32, kind="ExternalInput")
    x3_out = nc.dram_tensor("x3T", [C, T], f32r, kind="ExternalOutput")
    h2_out = nc.dram_tensor("h2T", [C, T], f32r, kind="ExternalOutput")

    with TileContext(nc) as tc, \
         nc.allow_low_precision("float32r rounding of matmul operands is intended"):
        with tc.tile_pool(name="acts", bufs=1) as acts, \
             tc.tile_pool(name="wpool", bufs=8) as wpool, \
             tc.tile_pool(name="vec", bufs=1) as vecp, \
             tc.tile_pool(name="sq", bufs=2) as sqp, \
             tc.tile_pool(name="ps_big", bufs=2, space="PSUM") as ps_big, \
             tc.tile_pool(name="ps_sc", bufs=2, space="PSUM") as ps_sc, \
             tc.tile_pool(name="ps_bc", bufs=1, space="PSUM") as ps_bc, \
             tc.tile_pool(name="ps_st", bufs=1, space="PSUM") as ps_st:

            evict_ctr = [0]

            def evict_copy(dst_ap, src_ap):
                # alternate psum->sbuf copies between ACT and DVE
                if evict_ctr[0] % 2 == 0:
                    nc.scalar.copy(dst_ap, src_ap)
                else:
                    nc.vector.tensor_copy(dst_ap, src_ap)
                evict_ctr[0] += 1

            # --- small constants / vectors ---
            ones_row = vecp.tile([1, T], f32r, tag="ones_row")
            nc.sync.dma_start(ones_row[:, :], din["onesr"][:, :])
            ones_col = vecp.tile([P, 1], f32r, tag="ones_col")
            nc.sync.dma_start(ones_col[:, :], din["onesc"][:, :])
            eps_t = vecp.tile([1, 1], f32, tag="eps")
            nc.vector.memset(eps_t[:, :], EPS)
            cols = vecp.tile([P, 7 * CT], f32, tag="cols")
            nc.sync.dma_start(cols[:, :], din["cols"][:, :])
            c_sc1m, c_shm = cols[:, 0:CT], cols[:, CT:2 * CT]
            c_gam, c_bet = cols[:, 2 * CT:3 * CT], cols[:, 3 * CT:4 * CT]
            c_sc1f, c_shf = cols[:, 4 * CT:5 * CT], cols[:, 5 * CT:6 * CT]
            gcol = cols[:, 6 * CT:7 * CT]
            r_bpsa = vecp.tile([1, C], f32r, tag="row_bpsa")
            nc.sync.dma_start(r_bpsa[:, :], din["rows2"][0:1, :])
            r_bpca = vecp.tile([1, C], f32r, tag="row_bpca")
            nc.sync.dma_start(r_bpca[:, :], din["rows2"][1:2, :])

            # --- load activations ---
            def load_fm(name, tag, ncols=T):
                ts = []
                for i in range(CT):
                    t_ = acts.tile([P, ncols], f32r, tag=f"{tag}{i}")
                    nc.sync.dma_start(t_[:, :], din[name][i * P:(i + 1) * P, :])
                    ts.append(t_)
                return ts

            x_t = load_fm("xT", "x")
            xo_t = load_fm("xoT", "xo")
            c_t = load_fm("cT", "c")

            def load_w(name):
                ws = []
                for ki in range(CT):
                    w = wpool.tile([P, C], f32r, tag="wblk")
                    nc.sync.dma_start(w[:, :], din[name][ki * P:(ki + 1) * P, :])
                    ws.append(w)
                return ws

            # --- LayerNorm helpers ---
            def ln_stats(src_tiles, label):
                """Return (a, b) [1,T] f32r tiles: a = rstd, b = -mean*rstd."""
                st_x = ps_st.tile([1, T], f32, tag="st")
                for i in range(CT):
                    nc.tensor.matmul(st_x[:, :], ones_col[:, :], src_tiles[i][:, :],
                                     start=(i == 0), stop=(i == CT - 1))
                mean = vecp.tile([1, T], f32, tag="mean", bufs=1)
                nc.vector.tensor_scalar_mul(mean[:, :], st_x[:, :], 1.0 / C)
                st_xx = ps_st.tile([1, T], f32, tag="st")
                for i in range(CT):
                    sq = sqp.tile([P, T], f32r, tag="sq")
                    nc.scalar.square(sq[:, :], src_tiles[i][:, :])
                    nc.tensor.matmul(st_xx[:, :], ones_col[:, :], sq[:, :],
                                     start=(i == 0), stop=(i == CT - 1))
                mean2 = vecp.tile([1, T], f32, tag="mean2", bufs=1)
                nc.vector.tensor_mul(mean2[:, :], mean[:, :], mean[:, :])
                var = vecp.tile([1, T], f32, tag="var", bufs=1)
                nc.vector.scalar_tensor_tensor(var[:, :], st_xx[:, :], 1.0 / C,
                                               mean2[:, :], ALU.mult, ALU.subtract)
                sd = vecp.tile([1, T], f32, tag="sd", bufs=1)
                nc.scalar.activation(sd[:, :], var[:, :], AF.Sqrt,
                                     bias=eps_t[:, 0:1])
                a = vecp.tile([1, T], f32r, tag="a", bufs=1)
                nc.vector.reciprocal(a[:, :], sd[:, :])
                b = vecp.tile([1, T], f32r, tag="b", bufs=1)
                nc.vector.scalar_tensor_tensor(b[:, :], mean[:, :], -1.0, a[:, :],
                                               ALU.mult, ALU.mult)
                return a, b

            def ln_apply(src_tiles, a, b, scale_cols, shift_cols, out_tag,
                         out_tiles=None):
                """out = (ln(src)) * scale[c] + shift[c], feature-major.

                scale_cols/shift_cols: [P, CT] f32 column packs (or None).
                """
                outs = []
                # the broadcasts are identical for every C-tile: compute once
                bcA = ps_bc.tile([P, T], f32, tag="bcA")
                nc.tensor.matmul(bcA[:, :], ones_row[:, 0:P], a[:, :],
                                 start=True, stop=True)
                bcB = ps_bc.tile([P, T], f32, tag="bcB")
                nc.tensor.matmul(bcB[:, :], ones_row[:, 0:P], b[:, :],
                                 start=True, stop=True)
                for i in range(CT):
                    if out_tiles is not None:
                        o = out_tiles[i]
                    else:
                        o = acts.tile([P, T], f32r, tag=f"{out_tag}{i}")
                    nc.vector.tensor_mul(o[:, :], src_tiles[i][:, :], bcA[:, :])
                    nc.vector.tensor_add(o[:, :], o[:, :], bcB[:, :])
                    if scale_cols is not None:
                        nc.vector.tensor_scalar(o[:, :], o[:, :],
                                                scale_cols[:, i:i + 1],
                                                shift_cols[:, i:i + 1],
                                                ALU.mult, ALU.add)
                    outs.append(o)
                return outs

            # --- LN1 + modulate (own + other half), LN on context ---
            a_x, b_x = ln_stats(x_t, "x")
            h_t = ln_apply(x_t, a_x, b_x, c_sc1m, c_shm, "h")
            a_xo, b_xo = ln_stats(xo_t, "xo")
            ho_t = ln_apply(xo_t, a_xo, b_xo, c_sc1m, c_shm, "ho")
            a_c, b_c = ln_stats(c_t, "c")
            cm_t = ln_apply(c_t, a_c, b_c, c_gam, c_bet, "cm")

            # --- linear helper: out_fm[oi] += W[:, oi].T @ act ---
            def linear_fm(w_tiles, act_tiles, out_tag, out_dtype=f32r,
                          bias_row=None, n_out=CT, evict=None):
                outs = []
                for oi in range(n_out):
                    ps = ps_big.tile([P, T], f32, tag="big")
                    for ki in range(CT):
                        nc.tensor.matmul(ps[:, :],
                                         w_tiles[ki][:, oi * P:(oi + 1) * P],
                                         act_tiles[ki][:, :],
                                         start=(ki == 0),
                                         stop=(ki == CT - 1 and bias_row is None))
                    if bias_row is not None:
                        nc.tensor.matmul(ps[:, :], bias_row[:, oi * P:(oi + 1) * P],
                                         ones_row[:, :], start=False, stop=True)
                    if evict is not None:
                        outs.append(evict(oi, ps))
                    else:
                        o = acts.tile([P, T], out_dtype, tag=f"{out_tag}{oi}")
                        evict_copy(o[:, :], ps[:, :])
                        outs.append(o)
                return outs

            # --- q/k/v projections ---
            w_q = load_w("Wq_sa")
            q_t = linear_fm(w_q, h_t, "q")

            w_k = load_w("Wk_sa")
            k_t = []
            for oi in range(CT):
                kt_ = acts.tile([P, N], f32r, tag=f"k{oi}")
                k_t.append(kt_)
            for half, act in ((0, h_t), (1, ho_t)):
                for oi in range(CT):
                    ps = ps_big.tile([P, T], f32, tag="big")
                    for ki in range(CT):
                        nc.tensor.matmul(ps[:, :], w_k[ki][:, oi * P:(oi + 1) * P],
                                         act[ki][:, :], start=(ki == 0),
                                         stop=(ki == CT - 1))
                    evict_copy(k_t[oi][:, half * T:(half + 1) * T], ps[:, :])

            # v in token-major interleaved layout: v_tm[kt][:, 65h:65h+64] = v,
            # col 65h+64 = ones  (for fused row-sum in the AV matmul)
            w_v = load_w("Wv_sa")
            v_tm = []
            for kt in range(NKT):
                vt = acts.tile([P, H * VW], f32r, tag=f"vtm{kt}")
                v_tm.append(vt)
                nc.sync.dma_start(vt[:, :], din["vinit"][:, :])
            for kt in range(NKT):
                act = h_t if kt < 4 else ho_t
                tj = kt % 4
                for ch in range(2):  # two 384-wide output chunks (6 heads each)
                    ps = ps_big.tile([P, 384], f32, tag="big")
                    for ki in range(CT):
                        nc.tensor.matmul(
                            ps[:, :],
                            act[ki][:, tj * P:(tj + 1) * P],
                            w_v[ki][:, ch * 384:(ch + 1) * 384],
                            start=(ki == 0), stop=(ki == CT - 1))
                    # strided eviction into the interleaved layout
                    vb = v_tm[kt][:, :]
                    dst = bass.AP(vb.tensor, vb.offset + ch * 6 * VW,
                                  [list(vb.ap[0]), [VW, 6], [1, 64]])
                    evict_copy(dst, ps[:, :].rearrange("p (h d) -> p h d", h=6))
                evict_ctr[0] += 1

            # --- generic attention: scores^T -> exp -> AV (+fused softmax norm) ---
            def attention(q_tiles, k_tiles, v_tm_tiles, nkt, out_tag):
                """Returns normalized attention output, feature-major [CT x [P,T]]."""
                out_tiles = [acts.tile([P, T], f32r, tag=f"{out_tag}{i}",
                                       name=f"{out_tag}{i}")
                             for i in range(CT)]
                for h in range(H):
                    th, ro = h // 2, 64 * (h % 2)
                    exps = []
                    for kt in range(nkt):
                        sps = ps_sc.tile([P, T], f32, tag="score")
                        nc.tensor.matmul(
                            sps[:, :],
                            k_tiles[th][ro:ro + 64, kt * P:(kt + 1) * P],
                            q_tiles[th][ro:ro + 64, :],
                            start=True, stop=True)
                        ex = acts.tile([P, T], f32r, tag=f"exp{kt}")
                        nc.scalar.activation(ex[:, :], sps[:, :], AF.Exp,
                                             scale=float(D ** -0.5))
                        exps.append(ex)
                    avps = ps_big.tile([VW, T], f32, tag="av", bufs=1)
                    for kt in range(nkt):
                        nc.tensor.matmul(avps[:, :],
                                         v_tm_tiles[kt][:, h * VW:(h + 1) * VW],
                                         exps[kt][:, :],
                                         start=(kt == 0), stop=(kt == nkt - 1))
                    # softmax denominator: recip of rowsum, broadcast to 64 rows,
                    # multiplied in during the psum eviction
                    srec = vecp.tile([1, T], f32r, tag="srec", bufs=2)
                    nc.vector.reciprocal(srec[:, :], avps[64:65, :])
                    brc = ps_bc.tile([64, T], f32, tag="bcA")
                    nc.tensor.matmul(brc[:, :], ones_row[:, 0:64], srec[:, :],
                                     start=True, stop=True)
                    evict_copy(out_tiles[th][ro:ro + 64, :], avps[0:64, :])
                    nc.vector.tensor_mul(out_tiles[th][ro:ro + 64, :],
                                         out_tiles[th][ro:ro + 64, :], brc[:, :])
                return out_tiles

            sa_t = attention(q_t, k_t, v_tm, NKT, "c")

            # --- proj_sa + gated residual: x2 = x + g_msa * (sa @ Wp + bp) ---
            w_p = load_w("Wp_sa")

            def evict_res_gated(oi, ps):
                nc.vector.scalar_tensor_tensor(x_t[oi][:, :], ps[:, :],
                                               gcol[:, oi:oi + 1], x_t[oi][:, :],
                                               ALU.mult, ALU.add)
                return x_t[oi]
            linear_fm(w_p, sa_t, None, bias_row=r_bpsa, evict=evict_res_gated)
            # x_t now holds x2

            # --- LN2 -> hx; q2 / gate projections ---
            a_2, b_2 = ln_stats(x_t, "x2")
            hx_t = ln_apply(x_t, a_2, b_2, None, None, "h", out_tiles=h_t)

            w_qq = load_w("Wqq")
            q2_t = linear_fm(w_qq, hx_t, "q")   # reuse q tags
            w_qg = load_w("Wqg")
            sig_t = []

            def evict_sig(oi, ps):
                o = acts.tile([P, T], f32, tag=f"xo{oi}")
                nc.scalar.activation(o[:, :], ps[:, :], AF.Sigmoid)
                sig_t.append(o)
                return o
            linear_fm(w_qg, hx_t, None, evict=evict_sig)

            # --- cross-attention K/V from modulated context ---
            w_k2 = load_w("Wk_ca")
            k2_t = linear_fm(w_k2, cm_t, "ho")
            w_v2 = load_w("Wv_ca")
            v2_tm = []
            for kt in range(LKT):
                vt = acts.tile([P, H * VW], f32r, tag=f"vtm{kt}")
                v2_tm.append(vt)
                nc.sync.dma_start(vt[:, :], din["vinit"][:, :])
            for kt in range(LKT):
                for ch in range(2):
                    ps = ps_big.tile([P, 384], f32, tag="big")
                    for ki in range(CT):
                        nc.tensor.matmul(
                            ps[:, :],
                            cm_t[ki][:, kt * P:(kt + 1) * P],
                            w_v2[ki][:, ch * 384:(ch + 1) * 384],
                            start=(ki == 0), stop=(ki == CT - 1))
                    vb = v2_tm[kt][:, :]
                    dst = bass.AP(vb.tensor, vb.offset + ch * 6 * VW,
                                  [list(vb.ap[0]), [VW, 6], [1, 64]])
                    evict_copy(dst, ps[:, :].rearrange("p (h d) -> p h d", h=6))
                evict_ctr[0] += 1

            ca_t = attention(q2_t, k2_t, v2_tm, LKT, "cm")
            for i in range(CT):
                nc.vector.tensor_mul(ca_t[i][:, :], ca_t[i][:, :], sig_t[i][:, :])

            # --- proj_ca + residual: x3 = x2 + (ca @ Wp_ca + bp_ca) ---
            w_p2 = load_w("Wp_ca")

            def evict_res(oi, ps):
                nc.vector.tensor_add(x_t[oi][:, :], ps[:, :], x_t[oi][:, :])
                nc.sync.dma_start(x3_out[oi * P:(oi + 1) * P, :], x_t[oi][:, :])
                return x_t[oi]
            linear_fm(w_p2, ca_t, None, bias_row=r_bpca, evict=evict_res)
            # x_t now holds x3

            # --- LN3 + MLP modulation -> h2 ---
            a_3, b_3 = ln_stats(x_t, "x3")
            h2_t = ln_apply(x_t, a_3, b_3, c_sc1f, c_shf, "h", out_tiles=h_t)
            for i in range(CT):
                nc.sync.dma_start(h2_out[i * P:(i + 1) * P, :], h2_t[i][:, :])

    nc.finalize()
    return nc


# --------------------------------------------------------------------------
# Pass B kernel builder: fp8 DoubleRow expert FFN over token bins.
#
# Per core: len(binsizes) bins; bin b holds tokens of ONE (virtual) expert,
# whose pre-scaled fp8 weights stream in per bin. Tokens are processed in
# 128-column chunks; each chunk's full h1 [F] lives across PSUM banks in two
# 12-plane halves so the gelu eviction runs as two wide ACT ops.
# --------------------------------------------------------------------------

FP8S = 64.0          # fp8 weight pre-scale (host multiplies W by this)
CHUNK = 128          # tokens per matmul chunk in pass B
PASS_B_CONFIGS = [(640, 640, 512), (768, 768, 512), (1024, 1024, 1024)]


def _build_pass_b(binsizes, with_bias):
    nc = bacc.Bacc("TRN2", target_bir_lowering=False, debug=False, num_devices=8)
    fp8 = mybir.dt.float8e4
    DR = mybir.MatmulPerfMode.DoubleRow
    NB = len(binsizes)
    BSMAX = max(binsizes)

    h2b = nc.dram_tensor("h2b", [NB, P, CT * BSMAX], fp8, kind="ExternalInput")
    w1b = nc.dram_tensor("w1b", [NB, P, CT * F], fp8, kind="ExternalInput")
    w2b = nc.dram_tensor("w2b", [NB, P, FT * C], fp8, kind="ExternalInput")
    if with_bias:
        b1c = nc.dram_tensor("b1c", [NB, P, FT], f32, kind="ExternalInput")
        b2c = nc.dram_tensor("b2c", [NB, P, CT], f32, kind="ExternalInput")
    y_out = nc.dram_tensor("y", [NB, P, CT * BSMAX], bf16, kind="ExternalOutput")

    with TileContext(nc) as tc:
        with tc.tile_pool(name="wp", bufs=2) as wp, \
             tc.tile_pool(name="hp", bufs=2) as hp, \
             tc.tile_pool(name="h1p", bufs=2) as h1p, \
             tc.tile_pool(name="yp", bufs=2) as yp, \
             tc.tile_pool(name="vec", bufs=2) as vecp, \
             tc.tile_pool(name="ps1", bufs=2, space="PSUM") as ps1p, \
             tc.tile_pool(name="psy", bufs=1, space="PSUM") as psyp:

            for b, BS in enumerate(binsizes):
                # h2 first (small), then w1 split per DoubleRow pair so the
                # first h1 matmuls start ~4us in; w2 streams under compute
                h2 = hp.tile([P, CT, BS], fp8, tag="h2")
                nc.sync.dma_start(
                    h2[:, :, :],
                    h2b[b].rearrange("p (k t) -> p k t", k=CT)[:, :, 0:BS])
                w1 = wp.tile([P, CT, F], fp8, tag="w1")
                w1d = w1b[b].rearrange("p (k f) -> p k f", k=CT)
                for pi in range(CT // 2):
                    nc.sync.dma_start(w1[:, 2 * pi:2 * pi + 2, :],
                                      w1d[:, 2 * pi:2 * pi + 2, :])
                w2 = wp.tile([P, FT, C], fp8, tag="w2")
                nc.sync.dma_start(w2[:, :, :],
                                  w2b[b].rearrange("p (k f) -> p k f", k=FT))
                if with_bias:
                    b1 = vecp.tile([P, FT], f32, tag="b1")
                    nc.sync.dma_start(b1[:, :], b1c[b, :, :])
                    b2 = vecp.tile([P, CT], f32, tag="b2")
                    nc.sync.dma_start(b2[:, :], b2c[b, :, :])
                yt = yp.tile([P, CT, BS], bf16, tag="y")

                nch = BS // CHUNK
                for ch in range(nch):
                    t0 = ch * CHUNK
                    rhs_h2 = h2[:, :, t0:t0 + CHUNK]
                    h1 = h1p.tile([P, FT, CHUNK], fp8, tag="h1")
                    for half in range(2):
                        psh = ps1p.tile([P, FT // 2, CHUNK], f32, tag="psh")
                        for oj in range(FT // 2):
                            fo = (half * (FT // 2) + oj) * P
                            for pi in range(CT // 2):
                                nc.tensor.matmul(
                                    psh[:, oj, :],
                                    w1[:, 2 * pi:2 * pi + 2, fo:fo + P],
                                    rhs_h2[:, 2 * pi:2 * pi + 2, :],
                                    start=(pi == 0), stop=(pi == CT // 2 - 1),
                                    perf_mode=DR)
                        dst = h1[:, half * (FT // 2):(half + 1) * (FT // 2), :]
                        if with_bias:
                            for oj in range(FT // 2):
                                ojg = half * (FT // 2) + oj
                                nc.scalar.activation(
                                    dst[:, oj, :], psh[:, oj, :], AF.Gelu,
                                    bias=b1[:, ojg:ojg + 1], scale=1.0 / FP8S)
                        else:
                            nc.scalar.activation(dst[:, :, :], psh[:, :, :],
                                                 AF.Gelu, scale=1.0 / FP8S)
                    psy = psyp.tile([P, CT, CHUNK], f32, tag="psy")
                    for oi in range(CT):
                        for pj in range(FT // 2):
                            nc.tensor.matmul(
                                psy[:, oi, :],
                                w2[:, 2 * pj:2 * pj + 2, oi * P:(oi + 1) * P],
                                h1[:, 2 * pj:2 * pj + 2, :],
                                start=(pj == 0), stop=(pj == FT // 2 - 1),
                                perf_mode=DR)
                    ydst = yt[:, :, t0:t0 + CHUNK]
                    if with_bias:
                        for oi in range(CT):
                            nc.vector.tensor_scalar(
                                ydst[:, oi, :], psy[:, oi, :], 1.0 / FP8S,
                                b2[:, oi:oi + 1], ALU.mult, ALU.add)
                    else:
                        nc.vector.tensor_scalar_mul(ydst[:, :, :],
                                                    psy[:, :, :], 1.0 / FP8S)
                # split export: bulk leaves while the last chunk computes
                y_dst = y_out[b].rearrange("p (k t) -> p k t", k=CT)
                nc.sync.dma_start(y_dst[:, :, 0:BS - CHUNK],
                                  yt[:, :, 0:BS - CHUNK])
                nc.sync.dma_start(y_dst[:, :, BS - CHUNK:BS],
                                  yt[:, :, BS - CHUNK:BS])

    nc.finalize()
    return nc


def _get_nc(which):
    if which not in _CACHE:
        if which == "a":
            _CACHE[which] = _build_pass_a()
        else:
            _, binsizes, with_bias = which
            _CACHE[which] = _build_pass_b(binsizes, with_bias)
    return _CACHE[which]


# --------------------------------------------------------------------------
# Host orchestration
# --------------------------------------------------------------------------

def _silu(x):
    return x / (1.0 + np.exp(-x))


def _softmax(x, axis=-1):
    x = x - x.max(axis=axis, keepdims=True)
    e = np.exp(x)
    return e / e.sum(axis=axis, keepdims=True)


def _ln_np(v, eps=EPS):
    m = v.mean(-1, keepdims=True)
    var = v.var(-1, keepdims=True)
    return (v - m) / np.sqrt(var + eps)


def _refine_logits(logits, amb, x, c, mod_vecs, tcond, W_qkv, Wqq, Wqg,
                   W_kv, Wp_sa, bp_sa, Wp_ca, bp_ca, W_router):
    """Recompute router logits exactly (fp32 host) for ambiguous tokens.

    The device pass runs matmuls in float32r (~11-bit mantissa), which is
    enough to route every token whose top-2 margin exceeds ~1e-4. For the
    handful of near-tie tokens, redo the whole block math for just those
    tokens in fp32 so the expert choice matches a full-precision reference.
    """
    f = np.float32
    sh_msa, sc_msa, g_msa, sh_mlp, sc_mlp, g_mlp, gamma, beta = mod_vecs
    scale = f(D) ** -0.5
    for b_ in np.unique(amb // N):
        tloc = amb[amb // N == b_] % N
        hb = _ln_np(x[b_]) * (1.0 + sc_msa[b_]) + sh_msa[b_]      # [N, C]
        k = (hb @ W_qkv[:, C:2 * C]).reshape(N, H, D)
        v = (hb @ W_qkv[:, 2 * C:]).reshape(N, H, D)
        q = (hb[tloc] @ W_qkv[:, :C]).reshape(-1, H, D)
        s = np.einsum('ahd,lhd->ahl', q * scale, k)
        s = np.exp(s - s.max(-1, keepdims=True))
        attn = s / s.sum(-1, keepdims=True)
        sa = np.einsum('ahl,lhd->ahd', attn, v).reshape(-1, C)
        sa = sa @ Wp_sa + bp_sa
        x2a = x[b_, tloc] + g_msa[b_] * sa
        cm = _ln_np(c[b_]) * gamma[b_] + beta[b_]
        k2 = (cm @ W_kv[:, :C]).reshape(L, H, D)
        v2 = (cm @ W_kv[:, C:]).reshape(L, H, D)
        hxa = _ln_np(x2a)
        q2 = (hxa @ Wqq).reshape(-1, H, D)
        gate = (hxa @ Wqg).reshape(-1, H, D)
        s2 = np.einsum('ahd,lhd->ahl', q2 * scale, k2)
        s2 = np.exp(s2 - s2.max(-1, keepdims=True))
        attn2 = s2 / s2.sum(-1, keepdims=True)
        ao = np.einsum('ahl,lhd->ahd', attn2, v2)
        ao = ao * (1.0 / (1.0 + np.exp(-gate)))
        ca = ao.reshape(-1, C) @ Wp_ca + bp_ca
        x3a = x2a + ca
        h2a = _ln_np(x3a) * (1.0 + sc_mlp[b_]) + sh_mlp[b_]
        logits[b_ * N + tloc] = h2a @ W_router + tcond[b_]
    return logits


def kernel(x, c, t, W_ada, b_ada, W_qkv, W_proj_sa, b_proj_sa, W_q, W_kv,
           W_proj_ca, b_proj_ca, W_cadaln, b_cadaln, W_router, W_tcond,
           W1, b1, W2, b2, Ws1, bs1, Ws2, bs2):
    f = np.float32
    x, c, t = np.asarray(x, f), np.asarray(c, f), np.asarray(t, f)

    # ---- host: tiny t-conditioned vectors (per batch) ----
    st = _silu(t)
    mod = st @ np.asarray(W_ada, f) + np.asarray(b_ada, f)          # [B, 6C]
    sh_msa, sc_msa, g_msa, sh_mlp, sc_mlp, g_mlp = np.split(mod, 6, axis=-1)
    gb = st @ np.asarray(W_cadaln, f) + np.asarray(b_cadaln, f)     # [B, 2C]
    gamma, beta = np.split(gb, 2, axis=-1)
    tcond = t @ np.asarray(W_tcond, f)                              # [B, E]

    # ---- pass A inputs ----
    W_qkv = np.asarray(W_qkv, f)
    Wq_sa = np.ascontiguousarray(W_qkv[:, :C])
    Wk_sa = np.ascontiguousarray(W_qkv[:, C:2 * C])
    Wv_sa = np.ascontiguousarray(W_qkv[:, 2 * C:])
    W_q = np.asarray(W_q, f).reshape(C, H, 2 * D)
    Wqq = np.ascontiguousarray(W_q[:, :, :D].reshape(C, C))
    Wqg = np.ascontiguousarray(W_q[:, :, D:].reshape(C, C))
    W_kv = np.asarray(W_kv, f)
    Wk_ca = np.ascontiguousarray(W_kv[:, :C])
    Wv_ca = np.ascontiguousarray(W_kv[:, C:])
    Wp_sa = np.asarray(W_proj_sa, f)
    Wp_ca = np.asarray(W_proj_ca, f)
    bp_sa = np.asarray(b_proj_sa, f)
    bp_ca = np.asarray(b_proj_ca, f)
    rows2 = np.ascontiguousarray(np.stack([bp_sa, bp_ca]))
    onesr = np.ones((1, T), f)
    onesc = np.ones((P, 1), f)
    vinit = np.zeros((P, H * VW), f)
    vinit[:, 64::VW] = 1.0

    in_maps_a = []
    for core in range(8):
        b_, half = core // 2, core % 2
        sl = slice(half * T, (half + 1) * T)
        so = slice((1 - half) * T, (2 - half) * T)
        cols = np.zeros((P, 7 * CT), f)
        for j, v in enumerate([1.0 + sc_msa[b_], sh_msa[b_], gamma[b_],
                               beta[b_], 1.0 + sc_mlp[b_], sh_mlp[b_],
                               g_msa[b_]]):
            cols[:, j * CT:(j + 1) * CT] = v.reshape(CT, P).T
        in_maps_a.append({
            "xT": np.ascontiguousarray(x[b_, sl].T),
            "xoT": np.ascontiguousarray(x[b_, so].T),
            "cT": np.ascontiguousarray(c[b_].T),
            "Wq_sa": Wq_sa, "Wk_sa": Wk_sa, "Wv_sa": Wv_sa,
            "Wqq": Wqq, "Wqg": Wqg, "Wk_ca": Wk_ca, "Wv_ca": Wv_ca,
            "Wp_sa": Wp_sa, "Wp_ca": Wp_ca,
            "rows2": rows2,
            "cols": cols,
            "onesr": onesr, "onesc": onesc, "vinit": vinit,
        })

    nc_a = _get_nc("a")
    res_a = run_bass_kernel_spmd(nc_a, in_maps_a, core_ids=list(range(8)))

    x3 = np.empty((B, N, C), f)
    h2 = np.empty((B, N, C), f)
    for core in range(8):
        b_, half = core // 2, core % 2
        sl = slice(half * T, (half + 1) * T)
        x3[b_, sl] = res_a.results[core]["x3T"].T
        h2[b_, sl] = res_a.results[core]["h2T"].T

    # ---- host: router (fp32) + top-2 + chunking ----
    W_router = np.asarray(W_router, f)
    h2f = h2.reshape(-1, C)
    logits = h2f @ W_router
    logits += np.repeat(tcond, N, axis=0)
    probs = _softmax(logits, axis=-1)
    # near-tie tokens: the device pass's float32r rounding could flip their
    # top-2 choice vs a full-precision reference -- redo just those on host
    ps_sorted = np.sort(probs, axis=-1)
    amb = np.nonzero(ps_sorted[:, -2] - ps_sorted[:, -3] < 2e-3)[0]
    if 0 < len(amb) <= 512:
        mod_vecs = (sh_msa, sc_msa, g_msa, sh_mlp, sc_mlp, g_mlp, gamma, beta)
        logits = _refine_logits(logits, amb, x, c, mod_vecs, tcond, W_qkv,
                                Wqq, Wqg, W_kv, Wp_sa, bp_sa, Wp_ca, bp_ca,
                                W_router)
        probs[amb] = _softmax(logits[amb], axis=-1)
    order = np.argsort(-probs, axis=-1, kind="stable")
    topi = order[:, :TOPK]
    topv = np.take_along_axis(probs, topi, axis=-1)
    topv = topv / topv.sum(-1, keepdims=True)

    W1 = np.asarray(W1, f)
    W2 = np.asarray(W2, f)
    b1 = np.asarray(b1, f)
    b2 = np.asarray(b2, f)
    Ws1 = np.asarray(Ws1, f)
    Ws2 = np.asarray(Ws2, f)
    bs1 = np.asarray(bs1, f)
    bs2 = np.asarray(bs2, f)
    with_bias = bool(b1.any() or b2.any() or bs1.any() or bs2.any())

    # ---- bin packing: 8 cores x NB bins; each bin = tokens of one expert ----
    # expert e token list (order arbitrary), shared pseudo-expert = E
    tok_by_e = [np.nonzero(topi == e_)[0] for e_ in range(E)]
    wv_by_e = [topv[topi == e_] for e_ in range(E)]
    all_toks = np.arange(B * N)
    tok_by_e.append(all_toks)
    wv_by_e.append(np.ones(B * N, f))

    def pack(binsizes):
        """Assign expert pieces to the 8*len(binsizes) bins (FFD).
        Returns list per global bin: (expert_id, tok_slice) or None."""
        nbins = 8 * len(binsizes)
        sizes = np.array([binsizes[i % len(binsizes)] for i in range(nbins)])
        order = np.argsort(-sizes, kind="stable")
        free = list(order)          # bin ids, largest first
        assign = [None] * nbins
        counts = sorted(range(E), key=lambda e_: -len(tok_by_e[e_]))
        for e_ in counts + [E]:
            toks = tok_by_e[e_]
            pos = 0
            while pos < len(toks):
                if not free:
                    return None
                # prefer the largest free bin; for the tail prefer smallest
                # bin that fits to save big bins for the shared expert
                rem = len(toks) - pos
                bid = None
                for j in range(len(free) - 1, -1, -1):
                    if sizes[free[j]] >= rem:
                        bid = free.pop(j)
                        break
                if bid is None:
                    bid = free.pop(0)
                take = min(rem, sizes[bid])
                assign[bid] = (e_, toks[pos:pos + take],
                               wv_by_e[e_][pos:pos + take])
                pos += take
        return assign

    assign = None
    for cfg in PASS_B_CONFIGS:
        assign = pack(list(cfg))
        if assign is not None:
            binsizes = list(cfg)
            break
    assert assign is not None, "no pass-B config fits this routing"
    global LAST_B_KEY
    LAST_B_KEY = ("b", tuple(binsizes), with_bias)

    NB = len(binsizes)
    BSMAX = max(binsizes)
    fp8np = ml_dtypes.float8_e4m3fn

    # pre-scaled fp8 weights, [P, CT*F] partition-major layout, per expert
    def prep_w(Wmat):  # [K, M] -> [P, (K//P)*M] with w[p, k, m] = W[k*P+p, m]
        Kd, Md = Wmat.shape
        return np.ascontiguousarray(
            (Wmat * FP8S).reshape(Kd // P, P, Md).transpose(1, 0, 2)
            .reshape(P, (Kd // P) * Md).astype(fp8np))

    w1_pre = [prep_w(W1[e_]) for e_ in range(E)] + [prep_w(Ws1)]
    w2_pre = [prep_w(W2[e_]) for e_ in range(E)] + [prep_w(Ws2)]
    b1_all = np.concatenate([b1, bs1[None]], 0)   # [E+1, F]
    b2_all = np.concatenate([b2, bs2[None]], 0)   # [E+1, C]

    # h2 tokens in fp8, partition-major [P, CT, B*N]
    h2q = h2f.astype(fp8np)                       # [B*N, C]
    h2pm = np.ascontiguousarray(
        h2q.T.reshape(CT, P, B * N).transpose(1, 0, 2))   # [P, CT, BN]

    in_maps_b = []
    for core in range(8):
        h2bin = np.zeros((NB, P, CT * BSMAX), fp8np)
        w1bin = np.empty((NB, P, CT * F), fp8np)
        w2bin = np.empty((NB, P, FT * C), fp8np)
        m = {"h2b": h2bin, "w1b": w1bin, "w2b": w2bin}
        if with_bias:
            m["b1c"] = np.zeros((NB, P, FT), f)
            m["b2c"] = np.zeros((NB, P, CT), f)
        for s in range(NB):
            a = assign[core * NB + s]
            BS = binsizes[s]
            if a is None:
                w1bin[s] = 0
                w2bin[s] = 0
                continue
            e_, toks, _wv = a
            w1bin[s] = w1_pre[e_]
            w2bin[s] = w2_pre[e_]
            h2bin[s].reshape(P, CT, BSMAX)[:, :, :len(toks)] = h2pm[:, :, toks]
            if with_bias:
                m["b1c"][s] = b1_all[e_].reshape(FT, P).T
                m["b2c"][s] = b2_all[e_].reshape(CT, P).T
        in_maps_b.append(m)

    nc_b = _get_nc(LAST_B_KEY)
    res_b = run_bass_kernel_spmd(nc_b, in_maps_b, core_ids=list(range(8)))

    # ---- host: weighted scatter-add combine + final residual ----
    accum = np.zeros((B * N, C), f)
    for core in range(8):
        y = np.asarray(res_b.results[core]["y"], f)  # [NB, P, CT*BSMAX]
        for s in range(NB):
            a = assign[core * NB + s]
            if a is None:
                continue
            e_, toks, wv = a
            # y[p, k, t] = out feature k*P+p of token t
            yv = y[s].reshape(P, CT, BSMAX)[:, :, :len(toks)]
            accum[toks] += wv[:, None] * yv.transpose(2, 1, 0).reshape(-1, C)

    out = x3 + g_mlp[:, None, :] * accum.reshape(B, N, C)
    return out.astype(np.float32)

